# revision 65
# baseline (speedup 1.0000x reference)
"""Multi-head attention (B=4, T=2048, D=1024, H=16) on 8 Trainium2 NeuronCores.

Sharding: core = (batch, head-group): b = core // 2, g = core % 2.
Each core computes heads [g*8, g*8+8) of batch b:
  - Q/K projections into transposed layout qT/kT = W_g @ x_b.T  [512, 2048]
  - V projection in natural layout [2048, 512] (+ ones column per head)
  - scores computed transposed: S.T tile = K_h @ Q_h.T on the PE; head pairs
    (2i, 2i+1) sit at partition bases 0/64 so their score matmuls run
    concurrently in distinct PE row groups
  - exp fused on ScalarE over two-bank PSUM groups (FD=1024), scale=1/sqrt(64),
    no max subtraction (logits ~N(0,1))
  - PV with lhsT = [V_h | 1] gives O.T[64, tq] and the softmax row-sums in row 64
  - normalize via reciprocal (reshaped across partitions) + partition-broadcast
  - partial output projection yT_g = Wo[:, g].T-contraction  [1024, 2048]
Host: y[b] = (yT_part[2b] + yT_part[2b+1]).T + bo.

The emission order software-pipelines the engines (head-pair-outer,
query-chunk-inner): Q/K m-tile prefetch is spread one n-chunk per window,
the V projection sits in pair 0, and the output projection trails per chunk
in pair 3, so the PE fills the gaps while ScalarE streams exp().

Self-contained: hardcodes all shapes; requires only concourse (bass) + numpy.
"""

import numpy as np

B, T, D = 4, 2048, 1024
H, HD = 16, 64
HG, DG = 8, 512          # heads / feature columns per core
NCORES = 8
P = 128
KD = D // P              # 8  k-tiles over model dim
MQ = DG // P             # 4  partition tiles of qT/kT/oT (one per head pair)
TK = T // P              # 16 key tiles
TQC = 512                # query-chunk (= one fp32 PSUM bank)
NC2 = T // TQC           # 4  query chunks
VW = HD + 1              # V columns per head incl. ones column
SCALE = 0.125            # 1/sqrt(HD)

_CACHE: dict = {}


def _emit(tc, aps, dbg=None, reps=1):
    import concourse.bass as bass  # noqa: F401
    from concourse import mybir

    nc = tc.nc
    dt = mybir.dt
    f32, bf16 = dt.float32, dt.bfloat16
    AF = mybir.ActivationFunctionType
    xT, wq, wk, wv, wo, bq, bk, bv, yT = (
        aps["xT"], aps["wq"], aps["wk"], aps["wv"], aps["wo"],
        aps["bq"], aps["bk"], aps["bv"], aps["yT"],
    )

    from contextlib import ExitStack

    with ExitStack() as ctx:
        const = ctx.enter_context(tc.tile_pool(name="const", bufs=1))
        persist = ctx.enter_context(tc.tile_pool(name="persist", bufs=1))
        xw = ctx.enter_context(tc.tile_pool(name="xw", bufs=1))
        ptp = ctx.enter_context(tc.tile_pool(name="ptp", bufs=4))
        pvsb = ctx.enter_context(tc.tile_pool(name="pvsb", bufs=2))
        yop = ctx.enter_context(tc.tile_pool(name="yop", bufs=3))
        nrm = ctx.enter_context(tc.tile_pool(name="nrm", bufs=2))
        scps = ctx.enter_context(tc.tile_pool(name="scps", bufs=2, space="PSUM"))
        qkvps = ctx.enter_context(tc.tile_pool(name="qkvps", bufs=2, space="PSUM"))
        pvps = ctx.enter_context(tc.tile_pool(name="pvps", bufs=2, space="PSUM"))

        # ---- persistent SBUF ----
        q_sb = persist.tile([P, MQ, T], bf16)
        k_sb = persist.tile([P, MQ, T], bf16)
        v_sb = persist.tile([P, TK, HG * VW], bf16)
        o_sb = persist.tile([P, MQ, T], bf16)
        v4d = v_sb.rearrange("p t (h c) -> p t h c", h=HG)
        nc.vector.memset(v4d[:, :, :, HD : HD + 1], 1.0)

        # ---- input DMAs: x on the SP queue, weights on the ACT queue so the
        # first q/k projections are not serialized behind the weight loads
        x_sb = xw.tile([P, KD, T], bf16)
        wq_sb = xw.tile([P, KD, DG], bf16)
        wk_sb = xw.tile([P, KD, DG], bf16)
        bq_sb = const.tile([P, MQ], f32)
        nc.sync.dma_start(out=bq_sb, in_=bq)
        bk_sb = const.tile([P, MQ], f32)
        nc.sync.dma_start(out=bk_sb, in_=bk)
        # wk leads the ACT queue (first PE op is its LDWEIGHTS), x is split
        # across both queues, wq follows the even x half on SP
        for ki in range(KD):
            nc.scalar.dma_start(out=wk_sb[:, ki], in_=wk[:, ki])
        for ki in range(0, KD, 2):
            nc.sync.dma_start(out=x_sb[:, ki], in_=xT[:, ki])
        for ki in range(1, KD, 2):
            nc.scalar.dma_start(out=x_sb[:, ki], in_=xT[:, ki])
        for ki in range(KD):
            nc.sync.dma_start(out=wq_sb[:, ki], in_=wq[:, ki])
        wv_sb = xw.tile([P, KD, DG], bf16)
        nc.scalar.dma_start(out=wv_sb, in_=wv)
        bv_sb = xw.tile([1, DG], bf16)
        nc.scalar.dma_start(out=bv_sb, in_=bv)
        ones_sb = xw.tile([1, P], bf16)
        nc.vector.memset(ones_sb, 1.0)
        ones_f32 = xw.tile([1, HD], f32)
        nc.vector.memset(ones_f32, 1.0)
        wo_sb = const.tile([P, MQ, D], bf16)
        nc.scalar.dma_start(out=wo_sb, in_=wo)

        def emit_qk_part(mt, n, which="kq"):
            """One T-chunk (n) of the q and/or k projection for m-tile mt."""
            sel = {
                "k": ((wk_sb, bk_sb, k_sb),),
                "q": ((wq_sb, bq_sb, q_sb),),
                "kq": ((wk_sb, bk_sb, k_sb), (wq_sb, bq_sb, q_sb)),
            }[which]
            # accumulate in x-arrival order (even k-tiles land first on SP)
            ki_order = list(range(0, KD, 2)) + list(range(1, KD, 2))
            for w_sb, b_col, dst in sel:
                ps = qkvps.tile([P, TQC], f32, tag="qkv", name="ps_qkv")
                for idx, ki in enumerate(ki_order):
                    nc.tensor.matmul(
                        ps,
                        w_sb[:, ki, mt * P : (mt + 1) * P],
                        x_sb[:, ki, n * TQC : (n + 1) * TQC],
                        start=(idx == 0),
                        stop=(idx == KD - 1),
                    )
                nc.vector.tensor_scalar_add(
                    dst[:, mt, n * TQC : (n + 1) * TQC], ps, b_col[:, mt : mt + 1]
                )

        def emit_v():
            for t in range(TK):
                ps = qkvps.tile([P, DG], f32, tag="qkv", name="ps_v")
                for ki in range(KD):
                    nc.tensor.matmul(
                        ps,
                        x_sb[:, ki, t * P : (t + 1) * P],
                        wv_sb[:, ki, :],
                        start=(ki == 0),
                        stop=False,
                    )
                nc.tensor.matmul(ps, ones_sb, bv_sb, start=False, stop=True)
                nc.vector.tensor_copy(
                    v4d[:, t, :, 0:HD], ps.rearrange("p (h c) -> p h c", h=HG)
                )

        def scores_exp_pair(p, c, pts):
            """Packed scores for heads (2p, 2p+1): concurrent PE row groups;
            exp over two-bank groups (FD = 2*TQC)."""
            tq0 = c * TQC
            for tkp in range(TK // 2):
                scs = [
                    scps.tile([P, 2, TQC], f32, tag="sc", name="sc0"),
                    scps.tile([P, 2, TQC], f32, tag="sc", name="sc1"),
                ]
                for u in range(2):
                    tk = 2 * tkp + u
                    for i in range(2):
                        hb = i * HD
                        nc.tensor.matmul(
                            scs[i][:, u, :],
                            k_sb[hb : hb + HD, p, tk * P : (tk + 1) * P],
                            q_sb[hb : hb + HD, p, tq0 : tq0 + TQC],
                            start=True,
                            stop=True,
                        )
                for i in range(2):
                    nc.scalar.activation(
                        pts[i][:, 2 * tkp : 2 * tkp + 2, :], scs[i], AF.Exp, scale=SCALE
                    )

        def pv_norm(p, c, i, pt):
            """PV + row-sum + normalize for head h = 2p + i."""
            h = 2 * p + i
            hb = i * HD
            tq0 = c * TQC
            pv = pvps.tile([VW, TQC], f32, name="pv")
            for tk in range(TK):
                nc.tensor.matmul(
                    pv,
                    v_sb[:, tk, h * VW : (h + 1) * VW],
                    pt[:, tk, :],
                    start=(tk == 0),
                    stop=(tk == TK - 1),
                )
            ps_o = pvsb.tile([VW, TQC], f32, name="ps_o")
            nc.vector.tensor_copy(ps_o, pv)
            rc = nrm.tile([1, TQC], f32, name="rc")
            nc.vector.reciprocal(rc, ps_o[HD : HD + 1, :])
            norm_pend.append((p, c, i, ps_o, rc))
            if dbg is not None and c == 0 and h == 0:
                nc.sync.dma_start(out=dbg["pt"], in_=pt)
                nc.sync.dma_start(out=dbg["pv"], in_=ps_o)
                nc.sync.dma_start(out=dbg["rc"], in_=rc)

        norm_pend = []

        def flush_norm():
            """Broadcast 1/rowsum across partitions with a K=1 ones matmul
            (PE), then multiply. Deferred one window behind the PV so the PE
            never waits on the reciprocal."""
            while norm_pend:
                p, c, i, ps_o, rc = norm_pend.pop(0)
                hb = i * HD
                tq0 = c * TQC
                bc = pvps.tile([HD, TQC], f32, tag="pv", name="bc")
                nc.tensor.matmul(bc, ones_f32, rc, start=True, stop=True)
                nc.vector.tensor_mul(
                    o_sb[hb : hb + HD, p, tq0 : tq0 + TQC], ps_o[0:HD, :], bc
                )

        def emit_oproj(c, alt_pool=False):
            tq0 = c * TQC
            yr = yT
            for j in range(D // P):
                # the final chunk runs after attention: the pv slots are idle,
                # so alternate pools for a 4-slot psum pipeline
                if alt_pool and j % 2 == 1:
                    ys = pvps.tile([P, TQC], f32, tag="pv", name="ys")
                else:
                    ys = qkvps.tile([P, TQC], f32, tag="qkv", name="ys")
                for ki in range(MQ):
                    nc.tensor.matmul(
                        ys,
                        wo_sb[:, ki, j * P : (j + 1) * P],
                        o_sb[:, ki, tq0 : tq0 + TQC],
                        start=(ki == 0),
                        stop=(ki == MQ - 1),
                    )
                yo = yop.tile([P, TQC], f32, name="yo")
                nc.vector.tensor_copy(yo, ys)
                nc.sync.dma_start(out=yr[:, j, tq0 : tq0 + TQC], in_=yo)

        # ---- schedule: pair-outer, chunk-inner ----
        if reps > 1:
            loop_cm = tc.For_i(0, reps, 1)
            loop_cm.__enter__()

        for n in range(NC2):
            emit_qk_part(0, n, "k")
        emit_qk_part(0, 0, "q")

        for p in range(MQ):
            for c in range(NC2):
                pts = [
                    ptp.tile([P, TK, TQC], bf16, tag="pt", name="pt0"),
                    ptp.tile([P, TK, TQC], bf16, tag="pt", name="pt1"),
                ]
                scores_exp_pair(p, c, pts)
                flush_norm()
                if p == 0 and c == 0:
                    emit_v()
                if p == 0 and c < NC2 - 1:
                    emit_qk_part(0, c + 1, "q")
                if p < MQ - 1:
                    emit_qk_part(p + 1, c, "kq")
                if p == MQ - 1 and c > 0:
                    emit_oproj(c - 1)
                pv_norm(p, c, 0, pts[0])
                pv_norm(p, c, 1, pts[1])
        flush_norm()
        emit_oproj(NC2 - 1, alt_pool=True)

        if reps > 1:
            loop_cm.__exit__(None, None, None)

        if dbg is not None:
            nc.sync.dma_start(out=dbg["q"], in_=q_sb)
            nc.sync.dma_start(out=dbg["k"], in_=k_sb)
            nc.sync.dma_start(out=dbg["v"], in_=v_sb)
            nc.sync.dma_start(out=dbg["o"], in_=o_sb)


def _build(debug=False, reps=1):
    import concourse.tile as tile
    from concourse import bacc, mybir

    dt = mybir.dt
    f32, bf16 = dt.float32, dt.bfloat16

    nc = bacc.Bacc("TRN2", target_bir_lowering=False, debug=False)
    # inputs are host-preswizzled into partition-major layouts so every DMA
    # descriptor is a fat contiguous run
    aps = {
        "xT": nc.dram_tensor("xT", [P, KD, T], bf16, kind="ExternalInput").ap(),
        "wq": nc.dram_tensor("wq", [P, KD, DG], bf16, kind="ExternalInput").ap(),
        "wk": nc.dram_tensor("wk", [P, KD, DG], bf16, kind="ExternalInput").ap(),
        "wv": nc.dram_tensor("wv", [P, KD, DG], bf16, kind="ExternalInput").ap(),
        "wo": nc.dram_tensor("wo", [P, MQ, D], bf16, kind="ExternalInput").ap(),
        "bq": nc.dram_tensor("bq", [P, MQ], f32, kind="ExternalInput").ap(),
        "bk": nc.dram_tensor("bk", [P, MQ], f32, kind="ExternalInput").ap(),
        "bv": nc.dram_tensor("bv", [1, DG], bf16, kind="ExternalInput").ap(),
        "yT": nc.dram_tensor("yT", [P, D // P, T], f32, kind="ExternalOutput").ap(),
    }

    dbg = None
    if debug:
        dbg = {
            "q": nc.dram_tensor("dbg_q", [P, MQ, T], bf16, kind="ExternalOutput").ap(),
            "k": nc.dram_tensor("dbg_k", [P, MQ, T], bf16, kind="ExternalOutput").ap(),
            "v": nc.dram_tensor(
                "dbg_v", [P, TK, HG * VW], bf16, kind="ExternalOutput"
            ).ap(),
            "o": nc.dram_tensor("dbg_o", [P, MQ, T], bf16, kind="ExternalOutput").ap(),
            "pt": nc.dram_tensor(
                "dbg_pt", [P, TK, TQC], bf16, kind="ExternalOutput"
            ).ap(),
            "pv": nc.dram_tensor("dbg_pv", [VW, TQC], f32, kind="ExternalOutput").ap(),
            "rc": nc.dram_tensor("dbg_rc", [1, TQC], f32, kind="ExternalOutput").ap(),
        }

    with tile.TileContext(nc) as tc:
        _emit(tc, aps, dbg, reps=reps)
    nc.compile()
    return nc


def _get_nc():
    if "nc" not in _CACHE:
        _CACHE["nc"] = _build()
    return _CACHE["nc"]


def _shard_inputs(x, Wq, bq, Wk, bk, Wv, bv, Wo, bo):
    import ml_dtypes

    bf16 = ml_dtypes.bfloat16
    f32 = np.float32

    def c(a, dtype):
        return np.ascontiguousarray(a).astype(dtype)

    def kp(a, kt):  # [kt*P, F] -> [P, kt, F] partition-major swizzle
        return a.reshape(kt, P, a.shape[-1]).transpose(1, 0, 2)

    in_maps = []
    for core in range(NCORES):
        b, g = core // 2, core % 2
        hs = g * DG
        in_maps.append(
            {
                "xT": c(kp(x[b].T, KD), bf16),
                "wq": c(kp(Wq[hs : hs + DG, :].T, KD), bf16),
                "wk": c(kp(Wk[hs : hs + DG, :].T, KD), bf16),
                "wv": c(kp(Wv[hs : hs + DG, :].T, KD), bf16),
                "wo": c(kp(Wo[:, hs : hs + DG].T, MQ), bf16),
                "bq": c(bq[hs : hs + DG].reshape(MQ, P).T, f32),
                "bk": c(bk[hs : hs + DG].reshape(MQ, P).T, f32),
                "bv": c(bv[hs : hs + DG].reshape(1, DG), bf16),
            }
        )
    return in_maps


def _run(inputs, trace=False):
    from concourse import bass_utils

    nc = _get_nc()
    np_in = {k: np.asarray(v) for k, v in inputs.items()}
    in_maps = _shard_inputs(**np_in)
    res = bass_utils.run_bass_kernel_spmd(
        nc, in_maps, core_ids=list(range(NCORES)), trace=trace
    )
    bo = np_in["bo"].astype(np.float32)
    y = np.empty((B, T, D), dtype=np.float32)
    for b in range(B):
        acc = res.results[2 * b]["yT"] + res.results[2 * b + 1]["yT"]  # [P, D/P, T]
        y[b] = acc.transpose(1, 0, 2).reshape(D, T).T + bo
    return y, res


def kernel(**inputs):
    y, _ = _run(inputs)
    return y
